# revision 6
# baseline (speedup 1.0000x reference)
"""Bass/Trainium2 kernel for nn_BasicModelThetaPerStep (GRU + per-step linears).

Self-contained: builds + compiles + runs the Bass program on 8 NeuronCores
(data parallel over batch), returns full outputs.

Model (see reference):
  B=256, T=512, DIN=65 (64 covariates + time feature), H=128, NDIST=3
  GRU over T steps, then:
    pred_params = exp(-(hs @ W_p.T + b_p))         [B, T, 3]
    hs (masked to zero past length)                [B, T, 128]
    cov = ([hs, x_time] @ W_c.T + b_c) masked      [B, T-1, 65] (last col zero)

Per-core device layout: [feature on partitions, (t, b) t-major on free dim].
"""
import os
import numpy as np

B, T, COV, H = 256, 512, 64, 128
DIN = COV + 1
NDIST = 3
NCORES = 8
BL = B // NCORES          # batch per core = 32
BT = T * BL               # free-dim grid per core = 16384
G3 = 3 * H                # 384

_CACHE = {}


def _build_program():
    import concourse.bass as bass
    import concourse.mybir as mybir
    import concourse.tile as tile
    from concourse import bacc
    from contextlib import ExitStack

    f32 = mybir.dt.float32
    AF = mybir.ActivationFunctionType
    OP = mybir.AluOpType

    nc = bacc.Bacc(None, target_bir_lowering=False)

    # ---- I/O ----
    xT = nc.dram_tensor("xT", [DIN + 1, BT], f32, kind="ExternalInput")
    h0T = nc.dram_tensor("h0T", [H, BL], f32, kind="ExternalInput")
    WihT = nc.dram_tensor("WihT", [DIN + 1, G3], f32, kind="ExternalInput")
    WhhT = nc.dram_tensor("WhhT", [H, G3], f32, kind="ExternalInput")
    bhn = nc.dram_tensor("bhn", [H, 1], f32, kind="ExternalInput")
    negbp = nc.dram_tensor("negbp", [NDIST, 1], f32, kind="ExternalInput")
    WpT = nc.dram_tensor("WpT", [H, NDIST], f32, kind="ExternalInput")
    WcT = nc.dram_tensor("WcT", [H, COV], f32, kind="ExternalInput")
    wct_bc = nc.dram_tensor("wct_bc", [2, COV], f32, kind="ExternalInput")

    hsT_out = nc.dram_tensor("hsT_out", [H, BT], f32, kind="ExternalOutput")
    predT = nc.dram_tensor("predT", [NDIST, BT], f32, kind="ExternalOutput")
    covT = nc.dram_tensor("covT", [COV, BT], f32, kind="ExternalOutput")

    NCHUNK = 32            # chunks of 512 free-cols (16 steps each)
    CW = BT // NCHUNK      # 512
    SPC = CW // BL         # steps per chunk = 16

    with tile.TileContext(nc) as tc, ExitStack() as ctx:
        singles = ctx.enter_context(tc.tile_pool(name="singles", bufs=1))
        xpool = ctx.enter_context(tc.tile_pool(name="xpool", bufs=4))
        gi_ps = ctx.enter_context(tc.tile_pool(name="gi_ps", bufs=2, space="PSUM"))
        dram = ctx.enter_context(tc.tile_pool(name="dram", bufs=1, space="DRAM"))
        gi_sb = ctx.enter_context(tc.tile_pool(name="gi_sb", bufs=8))
        step_ps = ctx.enter_context(tc.tile_pool(name="step_ps", bufs=2, space="PSUM"))
        work = ctx.enter_context(tc.tile_pool(name="work", bufs=4))
        out_ps = ctx.enter_context(tc.tile_pool(name="out_ps", bufs=2, space="PSUM"))
        out_sb = ctx.enter_context(tc.tile_pool(name="out_sb", bufs=4))

        # ---- Phase A: load constants ----
        w_ih = singles.tile([DIN + 1, G3], f32, tag="w_ih")
        nc.sync.dma_start(w_ih[:], WihT[:])
        w_hh = singles.tile([H, G3], f32, tag="w_hh")
        nc.sync.dma_start(w_hh[:], WhhT[:])
        b_hn = singles.tile([H, 1], f32, tag="b_hn")
        nc.sync.dma_start(b_hn[:], bhn[:])
        nbp = singles.tile([NDIST, 1], f32, tag="nbp")
        nc.sync.dma_start(nbp[:], negbp[:])
        w_p = singles.tile([H, NDIST], f32, tag="w_p")
        nc.sync.dma_start(w_p[:], WpT[:])
        w_c = singles.tile([H, COV], f32, tag="w_c")
        nc.sync.dma_start(w_c[:], WcT[:])
        w_cb = singles.tile([2, COV], f32, tag="w_cb")
        nc.sync.dma_start(w_cb[:], wct_bc[:])
        # time-feature row + ones row of xT, for the cov matmul rhs
        xt1 = singles.tile([2, BT], f32, tag="xt1")
        nc.sync.dma_start(xt1[:], xT[0:DIN + 1:DIN, :])

        # hs slab: col block t holds h_{t-1} (block 0 = h0); blocks 1..T = hs
        slab = singles.tile([H, (T + 1) * BL], f32, tag="slab")
        nc.sync.dma_start(slab[:, 0:BL], h0T[:])

        # ---- Phase B: gi = x_aug @ W_ih_aug^T for all steps, into DRAM ----
        # giD chunk c: [128, 16*96]; per step s: cols s*96+[r|z|n]
        giD = [dram.tile([H, SPC * 96], f32, tag=f"giD{c}", name=f"giD{c}")
               for c in range(NCHUNK)]
        for c in range(NCHUNK):
            xc = xpool.tile([DIN + 1, CW], f32, tag="xc")
            nc.sync.dma_start(xc[:], xT[:, c * CW:(c + 1) * CW])
            dst = giD[c][:].rearrange("p (s w) -> p s w", w=96)
            for g in range(3):
                gp = gi_ps.tile([H, CW], f32, tag="gp")
                nc.tensor.matmul(gp[:], w_ih[:, g * H:(g + 1) * H], xc[:],
                                 start=True, stop=True)
                gs = xpool.tile([H, CW], f32, tag="gs")
                if g % 2 == 0:
                    nc.scalar.copy(gs[:], gp[:])
                else:
                    nc.vector.tensor_copy(gs[:], gp[:])
                nc.sync.dma_start(dst[:, :, g * 32:(g + 1) * 32],
                                  gs[:].rearrange("p (s w) -> p s w", w=BL))

        # ---- Phase C: the scan ----
        for t in range(T):
            c, s = t // SPC, t % SPC
            h_prev = slab[:, t * BL:(t + 1) * BL]
            h_new = slab[:, (t + 1) * BL:(t + 2) * BL]

            gi_t = gi_sb.tile([H, 96], f32, tag="gi_t")
            nc.sync.dma_start(gi_t[:], giD[c][:, s * 96:(s + 1) * 96])

            ps = step_ps.tile([H, 96], f32, tag="ps")
            for g in range(3):
                nc.tensor.matmul(ps[:, g * 32:(g + 1) * 32],
                                 w_hh[:, g * H:(g + 1) * H], h_prev,
                                 start=True, stop=True)

            a_rz = work.tile([H, 64], f32, tag="a_rz")
            nc.vector.tensor_add(a_rz[:], ps[:, 0:64], gi_t[:, 0:64])
            rz = work.tile([H, 64], f32, tag="rz")
            nc.scalar.activation(rz[:], a_rz[:], AF.Sigmoid)
            # (h_n + b_hh_n) * r
            hnr = work.tile([H, 32], f32, tag="hnr")
            nc.vector.scalar_tensor_tensor(hnr[:], ps[:, 64:96], b_hn[:],
                                           rz[:, 0:32], OP.add, OP.mult)
            s_t = work.tile([H, 32], f32, tag="s_t")
            nc.vector.tensor_add(s_t[:], hnr[:], gi_t[:, 64:96])
            n_t = work.tile([H, 32], f32, tag="n_t")
            nc.scalar.activation(n_t[:], s_t[:], AF.Tanh)
            d_t = work.tile([H, 32], f32, tag="d_t")
            nc.vector.tensor_sub(d_t[:], h_prev, n_t[:])
            e_t = work.tile([H, 32], f32, tag="e_t")
            nc.vector.tensor_mul(e_t[:], rz[:, 32:64], d_t[:])
            nc.vector.tensor_add(h_new, n_t[:], e_t[:])

        # ---- Phase D: outputs ----
        hs_all = slab[:, BL:(T + 1) * BL]
        for c in range(NCHUNK):
            hs_c = hs_all[:, c * CW:(c + 1) * CW]
            # pred = exp(-(hs @ Wp^T + bp))
            pp = out_ps.tile([NDIST, CW], f32, tag="pp")
            nc.tensor.matmul(pp[:], w_p[:], hs_c, start=True, stop=True)
            pr = out_sb.tile([NDIST, CW], f32, tag="pr")
            nc.scalar.activation(pr[:], pp[:], AF.Exp, bias=nbp[:], scale=-1.0)
            nc.sync.dma_start(predT[:, c * CW:(c + 1) * CW], pr[:])
            # cov = hs @ Wc_h^T + xtime*w_t + b_c  (mask applied on host)
            cp = out_ps.tile([COV, CW], f32, tag="cp")
            nc.tensor.matmul(cp[:], w_c[:], hs_c, start=True, stop=False)
            nc.tensor.matmul(cp[:], w_cb[:], xt1[:, c * CW:(c + 1) * CW],
                             start=False, stop=True)
            cv = out_sb.tile([COV, CW], f32, tag="cv")
            nc.vector.tensor_copy(cv[:], cp[:])
            nc.sync.dma_start(covT[:, c * CW:(c + 1) * CW], cv[:])
        # hs out
        for i in range(8):
            w = BT // 8
            nc.sync.dma_start(hsT_out[:, i * w:(i + 1) * w],
                              hs_all[:, i * w:(i + 1) * w])

    nc.finalize()
    return nc


def _get_program():
    if "nc" not in _CACHE:
        _CACHE["nc"] = _build_program()
    return _CACHE["nc"]


def kernel(x, lengths, h0, W_ih, W_hh, b_ih, b_hh, W_p, b_p, W_c, b_c):
    from concourse.bass_utils import run_bass_kernel_spmd

    x = np.asarray(x, np.float32)
    lengths = np.asarray(lengths, np.int32)
    h0 = np.asarray(h0, np.float32)
    W_ih = np.asarray(W_ih, np.float32)
    W_hh = np.asarray(W_hh, np.float32)
    b_ih = np.asarray(b_ih, np.float32)
    b_hh = np.asarray(b_hh, np.float32)
    W_p = np.asarray(W_p, np.float32)
    b_p = np.asarray(b_p, np.float32)
    W_c = np.asarray(W_c, np.float32)
    b_c = np.asarray(b_c, np.float32)

    # shared (replicated) weight-derived inputs
    bias_fold = b_ih + np.concatenate([b_hh[:2 * H], np.zeros(H, np.float32)])
    WihT_aug = np.concatenate([W_ih.T, bias_fold[None, :]], axis=0)  # [66, 384]
    WhhT = np.ascontiguousarray(W_hh.T)                              # [128, 384]
    bhn = np.ascontiguousarray(b_hh[2 * H:].reshape(H, 1))
    negbp = np.ascontiguousarray(-b_p.reshape(NDIST, 1))
    WpT = np.ascontiguousarray(W_p.T)                                # [128, 3]
    WcT = np.ascontiguousarray(W_c[:, :H].T)                         # [128, 64]
    wct_bc = np.stack([W_c[:, H], b_c], axis=0)                      # [2, 64]

    in_maps = []
    for k in range(NCORES):
        sl = slice(k * BL, (k + 1) * BL)
        xc = x[sl]                                   # [32, 512, 65]
        xTc = xc.transpose(2, 1, 0).reshape(DIN, BT)  # [65, T*32] t-major
        xT_aug = np.concatenate([xTc, np.ones((1, BT), np.float32)], axis=0)
        in_maps.append({
            "xT": np.ascontiguousarray(xT_aug),
            "h0T": np.ascontiguousarray(h0[sl].T),
            "WihT": np.ascontiguousarray(WihT_aug),
            "WhhT": WhhT,
            "bhn": bhn,
            "negbp": negbp,
            "WpT": WpT,
            "WcT": WcT,
            "wct_bc": wct_bc,
        })

    nc = _get_program()
    res = run_bass_kernel_spmd(nc, in_maps, core_ids=list(range(NCORES)))
    _CACHE["in_maps"] = in_maps
    _CACHE["last_res"] = res

    hs = np.empty((B, T, H), np.float32)
    pred = np.empty((B, T, NDIST), np.float32)
    cov = np.empty((B, T - 1, COV + 1), np.float32)
    cov[:, :, COV] = 0.0
    for k in range(NCORES):
        sl = slice(k * BL, (k + 1) * BL)
        r = res.results[k]
        hs[sl] = r["hsT_out"].reshape(H, T, BL).transpose(2, 1, 0)
        pred[sl] = r["predT"].reshape(NDIST, T, BL).transpose(2, 1, 0)
        cov[sl, :, :COV] = r["covT"].reshape(COV, T, BL).transpose(2, 1, 0)[:, :T - 1]

    # exact mask repair (reference semantics)
    valid = (np.arange(T)[None, :] < lengths[:, None])
    hs *= valid[:, :, None]
    pred = np.where(valid[:, :, None], pred, np.exp(-b_p)[None, None, :])
    m2 = (np.arange(T - 1)[None, :] < (lengths - 1)[:, None])
    cov[:, :, :COV] *= m2[:, :, None]

    return pred.astype(np.float32), hs, cov


# revision 18
# speedup vs baseline: 1.4664x; 1.4664x over previous
"""Bass/Trainium2 kernel for nn_BasicModelThetaPerStep (GRU + per-step linears).

Self-contained: builds + compiles + runs the Bass program on 8 NeuronCores
(data parallel over batch), returns full outputs.

Model (see reference):
  B=256, T=512, DIN=65 (64 covariates + time feature), H=128, NDIST=3
  GRU over T steps, then:
    pred_params = exp(-(hs @ W_p.T + b_p))         [B, T, 3]
    hs (masked to zero past length)                [B, T, 128]
    cov = ([hs, x_time] @ W_c.T + b_c) masked      [B, T-1, 65] (last col zero)

Per-core device layout: [feature on partitions, (t, b) t-major on free dim].
"""
import os
import ml_dtypes
import numpy as np

BF16 = ml_dtypes.bfloat16

B, T, COV, H = 256, 512, 64, 128
DIN = COV + 1
NDIST = 3
NCORES = 8
BL = B // NCORES          # batch per core = 32
BT = T * BL               # free-dim grid per core = 16384
G3 = 3 * H                # 384

_CACHE = {}


def _build_program():
    import concourse.bass as bass
    import concourse.mybir as mybir
    import concourse.tile as tile
    from concourse import bacc
    from contextlib import ExitStack

    from concourse.masks import make_identity

    f32 = mybir.dt.float32
    bf16 = mybir.dt.bfloat16
    AF = mybir.ActivationFunctionType
    OP = mybir.AluOpType

    def mm(out, lhsT, rhs, **kw):
        nc.tensor.matmul(out, lhsT, rhs, **kw)

    nc = bacc.Bacc(None, target_bir_lowering=False)

    # ---- I/O ----
    xT = nc.dram_tensor("xT", [DIN + 1, BT], bf16, kind="ExternalInput")
    h0T = nc.dram_tensor("h0T", [H, BL], f32, kind="ExternalInput")
    WihT = nc.dram_tensor("WihT", [DIN + 1, G3], bf16, kind="ExternalInput")
    WhhT = nc.dram_tensor("WhhT", [H, G3], bf16, kind="ExternalInput")
    bhn = nc.dram_tensor("bhn", [H, 1], f32, kind="ExternalInput")
    negbp = nc.dram_tensor("negbp", [NDIST, 1], f32, kind="ExternalInput")
    WpT = nc.dram_tensor("WpT", [H, NDIST], f32, kind="ExternalInput")
    WcT = nc.dram_tensor("WcT", [H, COV], f32, kind="ExternalInput")
    wct_bc = nc.dram_tensor("wct_bc", [2, COV], bf16, kind="ExternalInput")

    hsT_out = nc.dram_tensor("hsT_out", [H, BT], f32, kind="ExternalOutput")
    predT = nc.dram_tensor("predT", [NDIST, BT], f32, kind="ExternalOutput")
    covT = nc.dram_tensor("covT", [COV, BT], f32, kind="ExternalOutput")

    NCHUNK = 32            # chunks of 512 free-cols (16 steps each)
    CW = BT // NCHUNK      # 512
    SPC = CW // BL         # steps per chunk = 16

    with tile.TileContext(nc) as tc, ExitStack() as ctx:
        singles = ctx.enter_context(tc.tile_pool(name="singles", bufs=1))
        xpool = ctx.enter_context(tc.tile_pool(name="xpool", bufs=4))
        gi_ps = ctx.enter_context(tc.tile_pool(name="gi_ps", bufs=2, space="PSUM"))
        dram = ctx.enter_context(tc.tile_pool(name="dram", bufs=1, space="DRAM"))
        gi_sb = ctx.enter_context(tc.tile_pool(name="gi_sb", bufs=8))
        step_ps = ctx.enter_context(tc.tile_pool(name="step_ps", bufs=2, space="PSUM"))
        work = ctx.enter_context(tc.tile_pool(name="work", bufs=4))
        out_ps = ctx.enter_context(tc.tile_pool(name="out_ps", bufs=2, space="PSUM"))
        out_sb = ctx.enter_context(tc.tile_pool(name="out_sb", bufs=4))

        # ---- Phase A: load constants ----
        w_ih = singles.tile([DIN + 1, G3], bf16, tag="w_ih")
        nc.sync.dma_start(w_ih[:], WihT[:])
        w_hh = singles.tile([H, G3], bf16, tag="w_hh")
        nc.sync.dma_start(w_hh[:], WhhT[:])
        b_hn = singles.tile([H, 1], f32, tag="b_hn")
        nc.sync.dma_start(b_hn[:], bhn[:])
        nbp = singles.tile([NDIST, 1], f32, tag="nbp")
        nc.sync.dma_start(nbp[:], negbp[:])
        w_p = singles.tile([H, NDIST], f32, tag="w_p")
        nc.sync.dma_start(w_p[:], WpT[:])
        w_c = singles.tile([H, COV], f32, tag="w_c")
        nc.sync.dma_start(w_c[:], WcT[:])
        w_cb = singles.tile([2, COV], bf16, tag="w_cb")
        nc.sync.dma_start(w_cb[:], wct_bc[:])


        # hs slab: col block t holds h_{t-1} (block 0 = h0); blocks 1..T = hs
        slab = singles.tile([H, (T + 1) * BL], f32, tag="slab")
        nc.sync.dma_start(slab[:, 0:BL], h0T[:])
        # identity for accumulating gi_rz into PSUM via PE
        ident = singles.tile([H, H], bf16, tag="ident")
        make_identity(nc, ident[:])
        # staging for pred pre-activations (one Exp at the end: avoids
        # sigmoid/exp ACT-table thrash during the scan)
        predS = singles.tile([NDIST, BT], f32, tag="predS")

        # ---- Phase B: gi = x_aug @ W_ih_aug^T for all steps, into DRAM ----
        # giD chunk c: [128, 16*96]; per step s: cols s*96+[r|z|n]
        giD = [dram.tile([H, SPC * 96], bf16, tag=f"giD{c}", name=f"giD{c}")
               for c in range(NCHUNK)]
        for c in range(NCHUNK):
            xc = xpool.tile([DIN + 1, CW], bf16, tag="xc")
            nc.sync.dma_start(xc[:], xT[:, c * CW:(c + 1) * CW])
            dst = giD[c][:].rearrange("p (s w) -> p s w", w=96)
            for g in range(3):
                gp = gi_ps.tile([H, CW], f32, tag="gp")
                mm(gp[:], w_ih[:, g * H:(g + 1) * H], xc[:],
                   start=True, stop=True)
                gs = xpool.tile([H, CW], bf16, tag="gs")
                if g % 2 == 0:
                    nc.scalar.copy(gs[:], gp[:])
                else:
                    nc.vector.tensor_copy(gs[:], gp[:])
                nc.sync.dma_start(dst[:, :, g * 32:(g + 1) * 32],
                                  gs[:].rearrange("p (s w) -> p s w", w=BL))

        # ---- Phase C: the scan ----
        GIC = 4  # steps per gi prefetch DMA
        hb_prev = work.tile([H, BL], bf16, tag="hb", bufs=3, name="hb_init")
        nc.vector.tensor_copy(hb_prev[:], slab[:, 0:BL])
        gi4 = None
        for t in range(T):
            c, s = t // SPC, t % SPC
            h_prev = slab[:, t * BL:(t + 1) * BL]
            h_new = slab[:, (t + 1) * BL:(t + 2) * BL]

            if t % GIC == 0:
                gi4 = gi_sb.tile([H, GIC * 96], bf16, tag="gi4", bufs=4)
                nc.sync.dma_start(gi4[:], giD[c][:, s * 96:(s + GIC) * 96])
            gi_t = gi4[:, (t % GIC) * 96:(t % GIC + 1) * 96]

            ps = step_ps.tile([H, 96], f32, tag="ps")
            mm(ps[:, 0:64], ident[:], gi_t[:, 0:64],
               start=True, stop=False)
            mm(ps[:, 0:32], w_hh[:, 0:H], hb_prev[:],
               start=False, stop=False, skip_group_check=True)
            mm(ps[:, 32:64], w_hh[:, H:2 * H], hb_prev[:],
               start=False, stop=True, skip_group_check=True)
            mm(ps[:, 64:96], w_hh[:, 2 * H:3 * H], hb_prev[:],
               start=True, stop=True)

            rz = work.tile([H, 64], f32, tag="rz")
            nc.scalar.activation(rz[:], ps[:, 0:64], AF.Sigmoid)
            # (h_n + b_hh_n) * r
            hnr = work.tile([H, 32], f32, tag="hnr")
            nc.vector.scalar_tensor_tensor(hnr[:], ps[:, 64:96], b_hn[:],
                                           rz[:, 0:32], OP.add, OP.mult)
            s_t = work.tile([H, 32], f32, tag="s_t")
            nc.vector.tensor_add(s_t[:], hnr[:], gi_t[:, 64:96])
            n_t = work.tile([H, 32], f32, tag="n_t")
            nc.scalar.activation(n_t[:], s_t[:], AF.Tanh)
            d_t = work.tile([H, 32], f32, tag="d_t")
            nc.vector.tensor_sub(d_t[:], h_prev, n_t[:])
            e_t = work.tile([H, 32], f32, tag="e_t")
            nc.vector.tensor_mul(e_t[:], rz[:, 32:64], d_t[:])
            # bf16 state for next step's matmuls (DVE, on the chain) and
            # fp32 slab write for hs/outputs (GpSimd, off the chain)
            hb_prev = work.tile([H, BL], bf16, tag="hb", bufs=3)
            nc.vector.tensor_add(hb_prev[:], n_t[:], e_t[:])
            nc.gpsimd.tensor_add(h_new, n_t[:], e_t[:])

        # ---- Phase D: outputs ----
        hs_all = slab[:, BL:(T + 1) * BL]
        for c in range(NCHUNK):
            hs_c = hs_all[:, c * CW:(c + 1) * CW]
            # pred pre-activation staged; single Exp at the end
            pp = out_ps.tile([NDIST, CW], f32, tag="pp")
            mm(pp[:], w_p[:], hs_c, start=True, stop=True)
            nc.scalar.copy(predS[:, c * CW:(c + 1) * CW], pp[:])
            # cov = hs @ Wc_h^T + xtime*w_t + b_c  (mask applied on host)
            xt1 = out_sb.tile([2, CW], bf16, tag="xt1")
            nc.sync.dma_start(xt1[:], xT[0:DIN + 1:DIN, c * CW:(c + 1) * CW])
            cp = out_ps.tile([COV, CW], f32, tag="cp")
            mm(cp[:], w_c[:], hs_c, start=True, stop=False)
            mm(cp[:], w_cb[:], xt1[:], start=False, stop=True)
            cv = out_sb.tile([COV, CW], f32, tag="cv")
            nc.vector.tensor_copy(cv[:], cp[:])
            nc.sync.dma_start(covT[:, c * CW:(c + 1) * CW], cv[:])
        for i in range(8):
            w = BT // 8
            pr = out_sb.tile([NDIST, w], f32, tag="pr", bufs=2)
            nc.scalar.activation(pr[:], predS[:, i * w:(i + 1) * w], AF.Exp,
                                 bias=nbp[:], scale=-1.0)
            nc.sync.dma_start(predT[:, i * w:(i + 1) * w], pr[:])
        # hs out
        for i in range(8):
            w = BT // 8
            nc.sync.dma_start(hsT_out[:, i * w:(i + 1) * w],
                              hs_all[:, i * w:(i + 1) * w])

    nc.finalize()
    return nc


def _get_program():
    if "nc" not in _CACHE:
        _CACHE["nc"] = _build_program()
    return _CACHE["nc"]


def kernel(x, lengths, h0, W_ih, W_hh, b_ih, b_hh, W_p, b_p, W_c, b_c):
    from concourse.bass_utils import run_bass_kernel_spmd

    x = np.asarray(x, np.float32)
    lengths = np.asarray(lengths, np.int32)
    h0 = np.asarray(h0, np.float32)
    W_ih = np.asarray(W_ih, np.float32)
    W_hh = np.asarray(W_hh, np.float32)
    b_ih = np.asarray(b_ih, np.float32)
    b_hh = np.asarray(b_hh, np.float32)
    W_p = np.asarray(W_p, np.float32)
    b_p = np.asarray(b_p, np.float32)
    W_c = np.asarray(W_c, np.float32)
    b_c = np.asarray(b_c, np.float32)

    # shared (replicated) weight-derived inputs
    bias_fold = b_ih + np.concatenate([b_hh[:2 * H], np.zeros(H, np.float32)])
    WihT_aug = np.concatenate([W_ih.T, bias_fold[None, :]], axis=0).astype(BF16)
    WhhT = np.ascontiguousarray(W_hh.T.astype(BF16))                 # [128, 384]
    bhn = np.ascontiguousarray(b_hh[2 * H:].reshape(H, 1))
    negbp = np.ascontiguousarray(-b_p.reshape(NDIST, 1))
    WpT = np.ascontiguousarray(W_p.T)                                # [128, 3]
    WcT = np.ascontiguousarray(W_c[:, :H].T)                         # [128, 64]
    wct_bc = np.stack([W_c[:, H], b_c], axis=0).astype(BF16)         # [2, 64]

    in_maps = []
    for k in range(NCORES):
        sl = slice(k * BL, (k + 1) * BL)
        xc = x[sl]                                   # [32, 512, 65]
        xTc = xc.transpose(2, 1, 0).reshape(DIN, BT)  # [65, T*32] t-major
        xT_aug = np.concatenate([xTc, np.ones((1, BT), np.float32)], axis=0)
        in_maps.append({
            "xT": np.ascontiguousarray(xT_aug.astype(BF16)),
            "h0T": np.ascontiguousarray(h0[sl].T),
            "WihT": np.ascontiguousarray(WihT_aug),
            "WhhT": WhhT,
            "bhn": bhn,
            "negbp": negbp,
            "WpT": WpT,
            "WcT": WcT,
            "wct_bc": wct_bc,
        })

    nc = _get_program()
    res = run_bass_kernel_spmd(nc, in_maps, core_ids=list(range(NCORES)))
    _CACHE["in_maps"] = in_maps
    _CACHE["last_res"] = res

    hs = np.empty((B, T, H), np.float32)
    pred = np.empty((B, T, NDIST), np.float32)
    cov = np.empty((B, T - 1, COV + 1), np.float32)
    cov[:, :, COV] = 0.0
    for k in range(NCORES):
        sl = slice(k * BL, (k + 1) * BL)
        r = res.results[k]
        hs[sl] = r["hsT_out"].reshape(H, T, BL).transpose(2, 1, 0)
        pred[sl] = r["predT"].reshape(NDIST, T, BL).transpose(2, 1, 0)
        cov[sl, :, :COV] = r["covT"].reshape(COV, T, BL).transpose(2, 1, 0)[:, :T - 1]

    # exact mask repair (reference semantics)
    valid = (np.arange(T)[None, :] < lengths[:, None])
    hs *= valid[:, :, None]
    pred = np.where(valid[:, :, None], pred, np.exp(-b_p)[None, None, :])
    m2 = (np.arange(T - 1)[None, :] < (lengths - 1)[:, None])
    cov[:, :, :COV] *= m2[:, :, None]

    return pred.astype(np.float32), hs, cov
